# revision 11
# baseline (speedup 1.0000x reference)
"""Differential multi-head cross-attention Trainium2 kernel (v2).

Sharding: 8 cores = 4 batches x 2 head-groups (8 heads each). Each core
computes its (batch, head-group) shard fully on device; the host sums the
two head-group partials per batch and adds the output bias.

v2 dataflow (all matmul operands bf16):
  qcT/kcT [d_cat=128, T] per head (q1/k1 rows 0-63, q2/k2 rows 64-127)
  scores psum [128 s, 2 sign, 512 t]; ONE 1024-wide exp per (sc, th)
  po FLIPPED: poT[t, 65] = e[s,t].T @ v_aug[s, 65]  (col 64 = Z)
    -> per-partition Z: reciprocal + tensor_scalar ops, no broadcasts
  w^T[t,d] = po1T*r1 - po2T*(lam*r2)  via ts + stt (accum_out = row sums)
  sumsq via tensor_tensor_reduce; partition sums via gpsimd all-reduce
  rsqrt via DVE pow(-0.5); ScalarE runs ONLY Exp/Copy (one act table)
  PE transpose w^T -> [d, t]; GN scale/bias fused into psum->sbuf copy
  y_part[t, e] = onorm.T @ out_w_slice
"""
import os
import sys

if "/opt/trn_rl_repo" not in sys.path:
    sys.path.insert(0, "/opt/trn_rl_repo")

import numpy as np

import concourse.bass as bass
import concourse.mybir as mybir
import concourse.tile as tile
from concourse import bacc
from concourse.bass import ts
from concourse.bass_utils import run_bass_kernel_spmd

F32 = mybir.dt.float32
BF16 = mybir.dt.bfloat16
AF = mybir.ActivationFunctionType
ALU = mybir.AluOpType

B, T, E, H, DH = 4, 1024, 1024, 16, 64
HL = 8          # local heads per core
EPS = 1e-5
NELEM = float(T * DH)   # groupnorm element count per (b, h)

MM_MODE = "bf16"
# Debug: stop after phase N (1=proj, 3=+scores/exp/po, 4=+wchain, 7=full)
PHASE = int(os.environ.get("KPHASE", "70"))

LAST_EXEC_NS = None
LAST_RESULTS = None


def build_module():
    nc = bacc.Bacc("TRN2", target_bir_lowering=False, debug=False, num_devices=8)
    mdt = BF16

    xT_d = nc.declare_dram_parameter("xT", [128, 8, T], mdt, isOutput=False)
    eT_d = nc.declare_dram_parameter("eT", [128, 8, T], mdt, isOutput=False)
    wq_d = nc.declare_dram_parameter("wq", [128, HL, 8, 128], mdt, isOutput=False)
    wk_d = nc.declare_dram_parameter("wk", [128, HL, 8, 128], mdt, isOutput=False)
    wv_d = nc.declare_dram_parameter("wv", [128, 8, 512], mdt, isOutput=False)
    qb_d = nc.declare_dram_parameter("qb", [128, HL], F32, isOutput=False)
    kb_d = nc.declare_dram_parameter("kb", [128, HL], F32, isOutput=False)
    vb_d = nc.declare_dram_parameter("vb", [128, 512], F32, isOutput=False)
    lamb_d = nc.declare_dram_parameter("lamb", [128, HL], F32, isOutput=False)
    gamp_d = nc.declare_dram_parameter("gamp", [128, 4], F32, isOutput=False)
    betp_d = nc.declare_dram_parameter("betp", [128, 4], F32, isOutput=False)
    ow_d = nc.declare_dram_parameter("ow", [128, 4, T], mdt, isOutput=False)
    vone_d = nc.declare_dram_parameter("vone", [128, 8, HL, 1], mdt, isOutput=False)
    idn_d = nc.declare_dram_parameter("idn", [128, 128], mdt, isOutput=False)
    y_d = nc.declare_dram_parameter("y", [T, E], F32, isOutput=True)

    def mm(ps, lhsT, rhs, start, stop):
        nc.tensor.matmul(ps, lhsT, rhs, start=start, stop=stop)

    with tile.TileContext(nc) as tc:
        with (
            tc.tile_pool(name="const", bufs=1) as cpool,
        ):
            # ---- constants / small tensors ----
            qb_sb = cpool.tile([128, HL], F32)
            nc.sync.dma_start(qb_sb, qb_d[:])
            kb_sb = cpool.tile([128, HL], F32)
            nc.sync.dma_start(kb_sb, kb_d[:])
            vb_sb = cpool.tile([128, 512], F32)
            nc.sync.dma_start(vb_sb, vb_d[:])
            lamb_sb = cpool.tile([128, HL], F32)
            nc.sync.dma_start(lamb_sb, lamb_d[:])
            gamp_sb = cpool.tile([128, 4], F32)
            nc.sync.dma_start(gamp_sb, gamp_d[:])
            betp_sb = cpool.tile([128, 4], F32)
            nc.sync.dma_start(betp_sb, betp_d[:])
            idn_sb = cpool.tile([128, 128], mdt)
            nc.sync.dma_start(idn_sb, idn_d[:])

            # Prime engine vector-clocks on the const DMAs so later compute
            # instructions don't each accumulate DMA-queue waits (ISA caps
            # the sync-wait count per instruction).
            prime_a = cpool.tile([128, 1], F32)
            nc.scalar.copy(prime_a, qb_sb[:, 0:1])
            nc.scalar.copy(prime_a, kb_sb[:, 0:1])
            prime_d = cpool.tile([128, 1], F32)
            nc.vector.tensor_copy(prime_d, vb_sb[:, 0:1])
            nc.vector.tensor_copy(prime_d, lamb_sb[:, 0:1])
            nc.vector.tensor_copy(prime_d, gamp_sb[:, 0:1])
            nc.vector.tensor_copy(prime_d, betp_sb[:, 0:1])
            prime_g = cpool.tile([128, 1], F32)
            nc.gpsimd.tensor_copy(prime_g, betp_sb[:, 0:1])

            # ---- persistent big tensors ----
            v_sb = cpool.tile([128, 8, HL, 65], mdt)     # [s_part, s_chunk, h, d|1]
            nc.sync.dma_start(v_sb[:, :, :, 64:65], vone_d[:])
            qc_sb = cpool.tile([128, HL, T], mdt)
            kc_sb = cpool.tile([128, HL, T], mdt)
            onorm = cpool.tile([128, 4, T], mdt)
            ow_sb = cpool.tile([128, 4, T], mdt)
            w_sb = cpool.tile([128, HL, 8, 64], mdt)     # [t_part, h, tc, d]
            stat_sb = cpool.tile([128, HL, 2, 8], F32)   # [t_part, h, {sum,sq}, tc]
            sar_sb = cpool.tile([128, HL, 2, 8], F32)    # all-reduced stats
            sums_sb = cpool.tile([128, HL, 2, 1], F32)
            chn_sb = cpool.tile([128, HL, 4], F32)       # mean, m2, mean^2, var
            rst_sb = cpool.tile([128, HL, 1], F32)       # rstd
            Gb_sb = cpool.tile([128, 4], F32)            # per-pair scale
            nB_sb = cpool.tile([128, 4], F32)            # per-pair mean*G - beta

            proj_pools = tc.tile_pool(name="bigin", bufs=1)
            bpool = proj_pools.__enter__()
            wpool_cm = tc.tile_pool(name="wts", bufs=1)
            wpool = wpool_cm.__enter__()
            epool_cm = tc.tile_pool(name="eps", bufs=1)
            epool = epool_cm.__enter__()
            spool_cm = tc.tile_pool(name="small", bufs=1)
            spool = spool_cm.__enter__()
            psum_cm = tc.tile_pool(name="psum", bufs=1, space="PSUM")
            psum = psum_cm.__enter__()

            xT_sb = bpool.tile([128, 8, T], mdt, tag="xT")
            eT_sb = bpool.tile([128, 8, T], mdt, tag="eT")
            wv_sb = bpool.tile([128, 8, 512], mdt, tag="wv")
            wq0 = wpool.tile([128, 8, 128], mdt, tag="wq", bufs=2)
            nc.sync.dma_start(wq0, wq_d[:, 0])
            wk0 = wpool.tile([128, 8, 128], mdt, tag="wk", bufs=2)
            nc.sync.dma_start(wk0, wk_d[:, 0])
            for o in range(8):
                nc.sync.dma_start(xT_sb[:, o], xT_d[:, o])
            for o in range(0, 8, 4):
                nc.sync.dma_start(eT_sb[:, o:o + 4], eT_d[:, o:o + 4])
            nc.sync.dma_start(wv_sb, wv_d[:])
            nc.sync.dma_start(ow_sb, ow_d[:])

            def proj_head(h, wq_t, wk_t):
                """q/k projections for head h (bf16 out with bias)."""
                for th in range(2):
                    pq = psum.tile([128, 512], F32, tag="pp", bufs=2)
                    for o in range(8):
                        mm(pq, wq_t[:, o], xT_sb[:, o, ts(th, 512)],
                           start=(o == 0), stop=(o == 7))
                    nc.vector.tensor_scalar_add(qc_sb[:, h, ts(th, 512)], pq,
                                                qb_sb[:, h:h + 1])
                    pk = psum.tile([128, 512], F32, tag="pp", bufs=2)
                    for o in range(8):
                        mm(pk, wk_t[:, o], eT_sb[:, o, ts(th, 512)],
                           start=(o == 0), stop=(o == 7))
                    nc.vector.tensor_scalar_add(kc_sb[:, h, ts(th, 512)], pk,
                                                kb_sb[:, h:h + 1])

            def proj_v(sc):
                """v projection chunk sc (natural [s, hd] layout)."""
                pv = psum.tile([128, 512], F32, tag="pp", bufs=2)
                for o in range(8):
                    mm(pv, eT_sb[:, o, ts(sc, 128)], wv_sb[:, o],
                       start=(o == 0), stop=(o == 7))
                nc.vector.tensor_tensor(
                    v_sb[:, sc, :, 0:64],
                    pv.rearrange("p (h d) -> p h d", d=64),
                    vb_sb.rearrange("p (h d) -> p h d", d=64),
                    ALU.add,
                )

            def load_w(h):
                if h == 0:
                    return wq0, wk0
                wq_t = wpool.tile([128, 8, 128], mdt, tag="wq", bufs=2)
                nc.sync.dma_start(wq_t, wq_d[:, h])
                wk_t = wpool.tile([128, 8, 128], mdt, tag="wk", bufs=2)
                nc.sync.dma_start(wk_t, wk_d[:, h])
                return wq_t, wk_t

            def attn_head(h):
                """scores + exp + flipped po + w-chain + stats for head h."""
                e_ts = []
                for th in range(2):
                    e_t = epool.tile([128, 8, 2, 512], mdt, tag="e", bufs=3)
                    e_ts.append(e_t)
                    for sc in range(8):
                        ps = psum.tile([128, 2, 512], F32, tag="sc", bufs=2)
                        mm(ps[:, 0], kc_sb[0:64, h, ts(sc, 128)],
                           qc_sb[0:64, h, ts(th, 512)], start=True, stop=True)
                        mm(ps[:, 1], kc_sb[64:128, h, ts(sc, 128)],
                           qc_sb[64:128, h, ts(th, 512)], start=True, stop=True)
                        nc.scalar.activation(e_t[:, sc], ps, AF.Exp, scale=0.125)
                if PHASE <= 2:
                    if h == 0:
                        nc.sync.dma_start(y_d[0:128, 0:512],
                                          e_ts[0][:, 0:1, :, :].bitcast(F32))
                    return
                for tc_i in range(8):
                    th, tj = divmod(tc_i, 4)
                    e_t = e_ts[th]
                    po = psum.tile([128, 2, 65], F32, tag="po", bufs=2)
                    for sg in range(2):
                        for sc in range(8):
                            mm(po[:, sg], e_t[:, sc, sg, ts(tj, 128)],
                               v_sb[:, sc, h], start=(sc == 0), stop=(sc == 7))
                    if PHASE <= 3:
                        if h == 0:
                            pd = spool.tile([128, 130], F32, tag="pd", bufs=2)
                            nc.vector.tensor_copy(
                                pd, po.rearrange("p a b -> p (a b)"))
                            nc.sync.dma_start(
                                y_d[ts(tc_i, 128), 0:130], pd)
                        continue
                    # w^T chain: r = 1/Z; m2 = po2*r2*lam; w = po1*r1 - m2
                    r = spool.tile([128, 2], F32, tag="r", bufs=2)
                    nc.vector.reciprocal(r, po[:, :, 64])
                    m2 = spool.tile([128, 64], F32, tag="m2", bufs=2)
                    nc.vector.tensor_scalar(m2, po[:, 1, 0:64], r[:, 1:2],
                                            lamb_sb[:, h:h + 1],
                                            ALU.mult, ALU.mult)
                    nc.vector.scalar_tensor_tensor(
                        w_sb[:, h, tc_i], po[:, 0, 0:64], r[:, 0:1], m2,
                        ALU.mult, ALU.subtract,
                        accum_out=stat_sb[:, h, 0, tc_i:tc_i + 1])
                    sqs = spool.tile([128, 64], mdt, tag="sqs", bufs=2)
                    nc.vector.tensor_tensor_reduce(
                        sqs, w_sb[:, h, tc_i], w_sb[:, h, tc_i], 1.0, 0.0,
                        ALU.mult, ALU.add,
                        accum_out=stat_sb[:, h, 1, tc_i:tc_i + 1])

                if PHASE <= 4:
                    if PHASE == 4:
                        nc.sync.dma_start(
                            y_d[ts(h, 128), 0:256],
                            w_sb[:, h].bitcast(F32).rearrange("p a b -> p (a b)"))
                    return

                # ---- groupnorm stats (per head) ----
                hp, hj = divmod(h, 2)
                rr = slice(64 * hj, 64 * hj + 64)
                nc.gpsimd.partition_all_reduce(
                    sar_sb[:, h], stat_sb[:, h], channels=128,
                    reduce_op=bass.bass_isa.ReduceOp.add)
                nc.vector.tensor_reduce(sums_sb[rr, h], sar_sb[rr, h],
                                        axis=mybir.AxisListType.X, op=ALU.add)
                nc.vector.tensor_scalar_mul(chn_sb[rr, h, 0:2],
                                            sums_sb[rr, h, :, 0], 1.0 / NELEM)
                nc.vector.tensor_tensor(chn_sb[rr, h, 2:3], chn_sb[rr, h, 0:1],
                                        chn_sb[rr, h, 0:1], ALU.mult)
                nc.vector.tensor_tensor(chn_sb[rr, h, 3:4], chn_sb[rr, h, 1:2],
                                        chn_sb[rr, h, 2:3], ALU.subtract)
                nc.vector.tensor_scalar(rst_sb[rr, h], chn_sb[rr, h, 3:4],
                                        EPS, -0.5, ALU.add, ALU.pow)
                nc.vector.tensor_tensor(Gb_sb[rr, hp:hp + 1], rst_sb[rr, h],
                                        gamp_sb[rr, hp:hp + 1], ALU.mult)
                nc.vector.scalar_tensor_tensor(
                    nB_sb[rr, hp:hp + 1], chn_sb[rr, h, 0:1],
                    Gb_sb[rr, hp:hp + 1], betp_sb[rr, hp:hp + 1],
                    ALU.mult, ALU.subtract)

            # ---- interleaved emission: proj h+1 then attention h ----
            proj_head(0, *load_w(0))
            for sc in range(4):
                proj_v(sc)
            for h in range(HL):
                if h + 1 < HL:
                    proj_head(h + 1, *load_w(h + 1))
                if h == 0:
                    for sc in range(4, 8):
                        proj_v(sc)
                if PHASE >= 2:
                    attn_head(h)

            if PHASE <= 1:
                for h in range(HL):
                    nc.sync.dma_start(y_d[ts(h, 128), 0:512],
                                      qc_sb[:, h, :].bitcast(F32))

            psum_cm.__exit__(None, None, None)
            spool_cm.__exit__(None, None, None)
            epool_cm.__exit__(None, None, None)
            wpool_cm.__exit__(None, None, None)
            proj_pools.__exit__(None, None, None)

            if PHASE >= 6:
                # ---- transpose + GN apply + final linear ----
                fpool_cm = tc.tile_pool(name="fin", bufs=1)
                fpool = fpool_cm.__enter__()
                fps_cm = tc.tile_pool(name="fpsum", bufs=1, space="PSUM")
                fps = fps_cm.__enter__()

                for hp in range(4):
                    tr = fps.tile([128, 8, 128], mdt, tag="tr", bufs=2)
                    for tc_i in range(8):
                        nc.tensor.transpose(tr[0:64, tc_i],
                                            w_sb[:, 2 * hp, tc_i], idn_sb)
                        nc.tensor.transpose(tr[64:128, tc_i],
                                            w_sb[:, 2 * hp + 1, tc_i], idn_sb)
                    for tc_i in range(8):
                        nc.vector.tensor_scalar(
                            onorm[:, hp, ts(tc_i, 128)], tr[:, tc_i],
                            Gb_sb[:, hp:hp + 1], nB_sb[:, hp:hp + 1],
                            ALU.mult, ALU.subtract)

                for tt_ in range(8 if PHASE >= 7 else 0):
                    for eh in range(2):
                        py = fps.tile([128, 512], F32, tag="fp", bufs=3)
                        for o in range(4):
                            mm(py, onorm[:, o, ts(tt_, 128)],
                               ow_sb[:, o, ts(eh, 512)],
                               start=(o == 0), stop=(o == 3))
                        yt = fpool.tile([128, 512], F32, tag="yt", bufs=3)
                        nc.scalar.copy(yt, py)
                        nc.sync.dma_start(y_d[ts(tt_, 128), ts(eh, 512)], yt)
                if PHASE == 6:
                    for g in range(4):
                        nc.sync.dma_start(
                            y_d[ts(g, 256), :].rearrange("(a p) t -> p (a t)", p=128),
                            onorm[:, g].bitcast(F32))
                fps_cm.__exit__(None, None, None)
                fpool_cm.__exit__(None, None, None)

    nc.finalize()
    return nc


_NC = None


def _get_nc():
    global _NC
    if _NC is None:
        _NC = build_module()
    return _NC


def _prep_core(c, x, eo, Wq_cat, Wk_cat, qb_cat, kb_cat, Vw, Vb, lam, gamr, betr,
               out_w, np_mdt):
    b, hg = divmod(c, 2)
    hs = slice(hg * 8, (hg + 1) * 8)

    def dev(a):
        return np.ascontiguousarray(a.astype(np_mdt))

    xT = x[b].T.reshape(8, 128, T).transpose(1, 0, 2)
    eT = eo[b].T.reshape(8, 128, T).transpose(1, 0, 2)
    wq = Wq_cat[hs].transpose(2, 0, 1).reshape(8, 128, HL, 128).transpose(1, 2, 0, 3)
    wk = Wk_cat[hs].transpose(2, 0, 1).reshape(8, 128, HL, 128).transpose(1, 2, 0, 3)
    wv = Vw[hs].reshape(512, E).T.reshape(8, 128, 512).transpose(1, 0, 2)
    ow = out_w[:, hg * 512:(hg + 1) * 512].T.reshape(4, 128, T).transpose(1, 0, 2)
    # pair-packed gamma/beta: rows 0:64 = even head of pair, 64:128 = odd
    gamp = np.concatenate([gamr[hs][0::2].T, gamr[hs][1::2].T], axis=0)  # [128,4]
    betp = np.concatenate([betr[hs][0::2].T, betr[hs][1::2].T], axis=0)
    return {
        "xT": dev(xT),
        "eT": dev(eT),
        "wq": dev(wq),
        "wk": dev(wk),
        "wv": dev(wv),
        "qb": np.ascontiguousarray(qb_cat[hs].T, dtype=np.float32),
        "kb": np.ascontiguousarray(kb_cat[hs].T, dtype=np.float32),
        "vb": np.ascontiguousarray(np.tile(Vb[hs].reshape(1, 512), (128, 1)),
                                   dtype=np.float32),
        "lamb": np.ascontiguousarray(np.tile(lam[hs][None, :], (128, 1)),
                                     dtype=np.float32),
        "gamp": np.ascontiguousarray(gamp, dtype=np.float32),
        "betp": np.ascontiguousarray(betp, dtype=np.float32),
        "ow": dev(ow),
    }


def make_in_maps(inputs):
    x = np.asarray(inputs["x"], np.float32)
    eo = np.asarray(inputs["encoder_out"], np.float32)
    Wq_cat = np.concatenate([np.asarray(inputs["Q1w"], np.float32),
                             np.asarray(inputs["Q2w"], np.float32)], axis=1)
    Wk_cat = np.concatenate([np.asarray(inputs["K1w"], np.float32),
                             np.asarray(inputs["K2w"], np.float32)], axis=1)
    qb_cat = np.concatenate([np.asarray(inputs["Q1b"], np.float32),
                             np.asarray(inputs["Q2b"], np.float32)], axis=1)
    kb_cat = np.concatenate([np.asarray(inputs["K1b"], np.float32),
                             np.asarray(inputs["K2b"], np.float32)], axis=1)
    Vw = np.asarray(inputs["Vw"], np.float32)
    Vb = np.asarray(inputs["Vb"], np.float32)
    lam = np.asarray(inputs["lam"], np.float32)
    gamr = np.asarray(inputs["gn_gamma"], np.float32).reshape(H, DH)
    betr = np.asarray(inputs["gn_beta"], np.float32).reshape(H, DH)
    out_w = np.asarray(inputs["out_w"], np.float32)

    import ml_dtypes
    np_mdt = ml_dtypes.bfloat16

    maps = [
        _prep_core(c, x, eo, Wq_cat, Wk_cat, qb_cat, kb_cat, Vw, Vb, lam,
                   gamr, betr, out_w, np_mdt)
        for c in range(8)
    ]
    for m in maps:
        m["vone"] = np.ones((128, 8, HL, 1), np_mdt)
        m["idn"] = np.eye(128, dtype=np_mdt)
    return maps


def kernel(**inputs):
    global LAST_EXEC_NS, LAST_RESULTS
    nc = _get_nc()
    in_maps = make_in_maps(inputs)
    res = run_bass_kernel_spmd(nc, in_maps, core_ids=list(range(8)))
    LAST_EXEC_NS = res.exec_time_ns
    LAST_RESULTS = res
    out_b = np.asarray(inputs["out_b"], np.float32)
    parts = [res.results[c]["y"] for c in range(8)]
    y = np.stack([parts[2 * b] + parts[2 * b + 1] for b in range(B)])
    y = y + out_b[None, None, :]
    return y.astype(np.float32)


# revision 15
# speedup vs baseline: 1.0164x; 1.0164x over previous
"""Differential multi-head cross-attention Trainium2 kernel (v2).

Sharding: 8 cores = 4 batches x 2 head-groups (8 heads each). Each core
computes its (batch, head-group) shard fully on device; the host sums the
two head-group partials per batch and adds the output bias.

v2 dataflow (all matmul operands bf16):
  qcT/kcT [d_cat=128, T] per head (q1/k1 rows 0-63, q2/k2 rows 64-127)
  scores psum [128 s, 2 sign, 512 t]; ONE 1024-wide exp per (sc, th)
  po FLIPPED: poT[t, 65] = e[s,t].T @ v_aug[s, 65]  (col 64 = Z)
    -> per-partition Z: reciprocal + tensor_scalar ops, no broadcasts
  w^T[t,d] = po1T*r1 - po2T*(lam*r2)  via ts + stt (accum_out = row sums)
  sumsq via tensor_tensor_reduce; partition sums via gpsimd all-reduce
  rsqrt via DVE pow(-0.5); ScalarE runs ONLY Exp/Copy (one act table)
  PE transpose w^T -> [d, t]; GN scale/bias fused into psum->sbuf copy
  y_part[t, e] = onorm.T @ out_w_slice
"""
import os
import sys

if "/opt/trn_rl_repo" not in sys.path:
    sys.path.insert(0, "/opt/trn_rl_repo")

import numpy as np

import concourse.bass as bass
import concourse.mybir as mybir
import concourse.tile as tile
from concourse import bacc
from concourse.bass import ts
from concourse.bass_utils import run_bass_kernel_spmd

F32 = mybir.dt.float32
BF16 = mybir.dt.bfloat16
FP8 = mybir.dt.float8e4
AF = mybir.ActivationFunctionType
ALU = mybir.AluOpType
DR = mybir.MatmulPerfMode.DoubleRow

SCORES_FP8 = True   # fp8e4m3 q/k + DoubleRow score matmuls

B, T, E, H, DH = 4, 1024, 1024, 16, 64
HL = 8          # local heads per core
EPS = 1e-5
NELEM = float(T * DH)   # groupnorm element count per (b, h)

MM_MODE = "bf16"
# Debug: stop after phase N (1=proj, 3=+scores/exp/po, 4=+wchain, 7=full)
PHASE = int(os.environ.get("KPHASE", "70"))

LAST_EXEC_NS = None
LAST_RESULTS = None


def build_module():
    nc = bacc.Bacc("TRN2", target_bir_lowering=False, debug=False, num_devices=8)
    mdt = BF16

    xT_d = nc.declare_dram_parameter("xT", [128, 8, T], mdt, isOutput=False)
    eT_d = nc.declare_dram_parameter("eT", [128, 8, T], mdt, isOutput=False)
    wq_d = nc.declare_dram_parameter("wq", [128, HL, 8, 128], mdt, isOutput=False)
    wk_d = nc.declare_dram_parameter("wk", [128, HL, 8, 128], mdt, isOutput=False)
    wv_d = nc.declare_dram_parameter("wv", [128, 8, 512], mdt, isOutput=False)
    qb_d = nc.declare_dram_parameter("qb", [128, HL], F32, isOutput=False)
    kb_d = nc.declare_dram_parameter("kb", [128, HL], F32, isOutput=False)
    vb_d = nc.declare_dram_parameter("vb", [128, 512], F32, isOutput=False)
    lamb_d = nc.declare_dram_parameter("lamb", [128, HL], F32, isOutput=False)
    gamp_d = nc.declare_dram_parameter("gamp", [128, 4], F32, isOutput=False)
    betp_d = nc.declare_dram_parameter("betp", [128, 4], F32, isOutput=False)
    ow_d = nc.declare_dram_parameter("ow", [128, 4, T], mdt, isOutput=False)
    vone_d = nc.declare_dram_parameter("vone", [128, 8, HL, 1], mdt, isOutput=False)
    idn_d = nc.declare_dram_parameter("idn", [128, 128], mdt, isOutput=False)
    y_d = nc.declare_dram_parameter("y", [T, E], F32, isOutput=True)

    def mm(ps, lhsT, rhs, start, stop):
        nc.tensor.matmul(ps, lhsT, rhs, start=start, stop=stop)

    with tile.TileContext(nc) as tc:
        with (
            tc.tile_pool(name="const", bufs=1) as cpool,
        ):
            # ---- constants / small tensors ----
            qb_sb = cpool.tile([128, HL], F32)
            nc.sync.dma_start(qb_sb, qb_d[:])
            kb_sb = cpool.tile([128, HL], F32)
            nc.sync.dma_start(kb_sb, kb_d[:])
            vb_sb = cpool.tile([128, 512], F32)
            nc.sync.dma_start(vb_sb, vb_d[:])
            lamb_sb = cpool.tile([128, HL], F32)
            nc.sync.dma_start(lamb_sb, lamb_d[:])
            gamp_sb = cpool.tile([128, 4], F32)
            nc.sync.dma_start(gamp_sb, gamp_d[:])
            betp_sb = cpool.tile([128, 4], F32)
            nc.sync.dma_start(betp_sb, betp_d[:])
            idn_sb = cpool.tile([128, 128], mdt)
            nc.sync.dma_start(idn_sb, idn_d[:])

            # Prime engine vector-clocks on the const DMAs so later compute
            # instructions don't each accumulate DMA-queue waits (ISA caps
            # the sync-wait count per instruction).
            prime_a = cpool.tile([128, 1], F32)
            nc.scalar.copy(prime_a, qb_sb[:, 0:1])
            nc.scalar.copy(prime_a, kb_sb[:, 0:1])
            prime_d = cpool.tile([128, 1], F32)
            nc.vector.tensor_copy(prime_d, vb_sb[:, 0:1])
            nc.vector.tensor_copy(prime_d, lamb_sb[:, 0:1])
            nc.vector.tensor_copy(prime_d, gamp_sb[:, 0:1])
            nc.vector.tensor_copy(prime_d, betp_sb[:, 0:1])
            prime_g = cpool.tile([128, 1], F32)
            nc.gpsimd.tensor_copy(prime_g, betp_sb[:, 0:1])

            # ---- persistent big tensors ----
            v_sb = cpool.tile([128, 8, HL, 65], mdt)     # [s_part, s_chunk, h, d|1]
            nc.sync.dma_start(v_sb[:, :, :, 64:65], vone_d[:])
            qk_dt = FP8 if SCORES_FP8 else mdt
            qc_sb = cpool.tile([128, HL, T], qk_dt)
            kc_sb = cpool.tile([128, HL, T], qk_dt)
            onorm = cpool.tile([128, 4, T], mdt)
            ow_sb = cpool.tile([128, 4, T], mdt)
            w_sb = cpool.tile([128, HL, 8, 64], mdt)     # [t_part, h, tc, d]
            stat_sb = cpool.tile([128, HL, 2, 8], F32)   # [t_part, h, {sum,sq}, tc]
            sar_sb = cpool.tile([128, HL, 2, 8], F32)    # all-reduced stats
            sums_sb = cpool.tile([128, HL, 2, 1], F32)
            chn_sb = cpool.tile([128, HL, 4], F32)       # mean, m2, mean^2, var
            rst_sb = cpool.tile([128, HL, 1], F32)       # rstd
            Gb_sb = cpool.tile([128, 4], F32)            # per-pair scale
            nB_sb = cpool.tile([128, 4], F32)            # per-pair mean*G - beta

            proj_pools = tc.tile_pool(name="bigin", bufs=1)
            bpool = proj_pools.__enter__()
            wpool_cm = tc.tile_pool(name="wts", bufs=1)
            wpool = wpool_cm.__enter__()
            epool_cm = tc.tile_pool(name="eps", bufs=1)
            epool = epool_cm.__enter__()
            spool_cm = tc.tile_pool(name="small", bufs=1)
            spool = spool_cm.__enter__()
            psum_cm = tc.tile_pool(name="psum", bufs=1, space="PSUM")
            psum = psum_cm.__enter__()

            xT_sb = bpool.tile([128, 8, T], mdt, tag="xT")
            eT_sb = bpool.tile([128, 8, T], mdt, tag="eT")
            wv_sb = bpool.tile([128, 8, 512], mdt, tag="wv")
            wq0 = wpool.tile([128, 8, 128], mdt, tag="wq", bufs=2)
            nc.sync.dma_start(wq0, wq_d[:, 0])
            wk0 = wpool.tile([128, 8, 128], mdt, tag="wk", bufs=2)
            nc.sync.dma_start(wk0, wk_d[:, 0])
            for o in range(8):
                nc.sync.dma_start(xT_sb[:, o], xT_d[:, o])
            for o in range(0, 8, 4):
                nc.sync.dma_start(eT_sb[:, o:o + 4], eT_d[:, o:o + 4])
            nc.sync.dma_start(wv_sb, wv_d[:])
            nc.sync.dma_start(ow_sb, ow_d[:])

            def proj_head(h, wq_t, wk_t):
                """q/k projections for head h (bf16 out with bias)."""
                for th in range(2):
                    pq = psum.tile([128, 512], F32, tag="pp", bufs=2)
                    for o in range(8):
                        mm(pq, wq_t[:, o], xT_sb[:, o, ts(th, 512)],
                           start=(o == 0), stop=(o == 7))
                    nc.vector.tensor_scalar_add(qc_sb[:, h, ts(th, 512)], pq,
                                                qb_sb[:, h:h + 1])
                    pk = psum.tile([128, 512], F32, tag="pp", bufs=2)
                    for o in range(8):
                        mm(pk, wk_t[:, o], eT_sb[:, o, ts(th, 512)],
                           start=(o == 0), stop=(o == 7))
                    nc.vector.tensor_scalar_add(kc_sb[:, h, ts(th, 512)], pk,
                                                kb_sb[:, h:h + 1])

            def proj_v(sc):
                """v projection chunk sc (natural [s, hd] layout)."""
                pv = psum.tile([128, 512], F32, tag="pp", bufs=2)
                for o in range(8):
                    mm(pv, eT_sb[:, o, ts(sc, 128)], wv_sb[:, o],
                       start=(o == 0), stop=(o == 7))
                nc.vector.tensor_tensor(
                    v_sb[:, sc, :, 0:64],
                    pv.rearrange("p (h d) -> p h d", d=64),
                    vb_sb.rearrange("p (h d) -> p h d", d=64),
                    ALU.add,
                )

            def load_w(h):
                if h == 0:
                    return wq0, wk0
                wq_t = wpool.tile([128, 8, 128], mdt, tag="wq", bufs=2)
                nc.sync.dma_start(wq_t, wq_d[:, h])
                wk_t = wpool.tile([128, 8, 128], mdt, tag="wk", bufs=2)
                nc.sync.dma_start(wk_t, wk_d[:, h])
                return wq_t, wk_t

            def attn_head(h):
                """scores + exp + flipped po + w-chain + stats for head h."""
                e_ts = []
                for th in range(2):
                    e_t = epool.tile([128, 8, 2, 512], mdt, tag="e", bufs=3)
                    e_ts.append(e_t)
                    for sc in range(8):
                        ps = psum.tile([128, 2, 512], F32, tag="sc", bufs=2)
                        if SCORES_FP8:
                            # DoubleRow with both k-tiles aliased (stride-0)
                            # computes 2*(k.T @ q); exp scale absorbs the 2x.
                            for sg in range(2):
                                pr = slice(64 * sg, 64 * sg + 64)
                                kb_ap = kc_sb[pr, h, ts(sc, 128)] \
                                    .unsqueeze(1).broadcast_to([64, 2, 128])
                                for tq in range(2):
                                    t0 = th * 512 + tq * 256
                                    qb_ap = qc_sb[pr, h, t0:t0 + 256] \
                                        .unsqueeze(1).broadcast_to([64, 2, 256])
                                    nc.tensor.matmul(
                                        ps[:, sg, ts(tq, 256)], kb_ap, qb_ap,
                                        start=True, stop=True, perf_mode=DR)
                        else:
                            mm(ps[:, 0], kc_sb[0:64, h, ts(sc, 128)],
                               qc_sb[0:64, h, ts(th, 512)], start=True, stop=True)
                            mm(ps[:, 1], kc_sb[64:128, h, ts(sc, 128)],
                               qc_sb[64:128, h, ts(th, 512)], start=True, stop=True)
                        nc.scalar.activation(e_t[:, sc], ps, AF.Exp,
                                             scale=0.0625 if SCORES_FP8 else 0.125)
                if PHASE <= 2:
                    if h == 0:
                        nc.sync.dma_start(y_d[0:128, 0:512],
                                          e_ts[0][:, 0:1, :, :].bitcast(F32))
                    return
                for tc_i in range(8):
                    th, tj = divmod(tc_i, 4)
                    e_t = e_ts[th]
                    po = psum.tile([128, 2, 65], F32, tag="po", bufs=2)
                    for sg in range(2):
                        for sc in range(8):
                            mm(po[:, sg], e_t[:, sc, sg, ts(tj, 128)],
                               v_sb[:, sc, h], start=(sc == 0), stop=(sc == 7))
                    if PHASE <= 3:
                        if h == 0:
                            pd = spool.tile([128, 130], F32, tag="pd", bufs=2)
                            nc.vector.tensor_copy(
                                pd, po.rearrange("p a b -> p (a b)"))
                            nc.sync.dma_start(
                                y_d[ts(tc_i, 128), 0:130], pd)
                        continue
                    # w^T chain: r = 1/Z; m2 = po2*r2*lam; w = po1*r1 - m2
                    r = spool.tile([128, 2], F32, tag="r", bufs=2)
                    nc.vector.reciprocal(r, po[:, :, 64])
                    m2 = spool.tile([128, 64], F32, tag="m2", bufs=2)
                    nc.vector.tensor_scalar(m2, po[:, 1, 0:64], r[:, 1:2],
                                            lamb_sb[:, h:h + 1],
                                            ALU.mult, ALU.mult)
                    nc.vector.scalar_tensor_tensor(
                        w_sb[:, h, tc_i], po[:, 0, 0:64], r[:, 0:1], m2,
                        ALU.mult, ALU.subtract,
                        accum_out=stat_sb[:, h, 0, tc_i:tc_i + 1])
                    sqs = spool.tile([128, 64], mdt, tag="sqs", bufs=2)
                    nc.vector.tensor_tensor_reduce(
                        sqs, w_sb[:, h, tc_i], w_sb[:, h, tc_i], 1.0, 0.0,
                        ALU.mult, ALU.add,
                        accum_out=stat_sb[:, h, 1, tc_i:tc_i + 1])

                if PHASE <= 4:
                    if PHASE == 4:
                        nc.sync.dma_start(
                            y_d[ts(h, 128), 0:256],
                            w_sb[:, h].bitcast(F32).rearrange("p a b -> p (a b)"))
                    return

                # ---- groupnorm stats (per head) ----
                hp, hj = divmod(h, 2)
                rr = slice(64 * hj, 64 * hj + 64)
                nc.gpsimd.partition_all_reduce(
                    sar_sb[:, h], stat_sb[:, h], channels=128,
                    reduce_op=bass.bass_isa.ReduceOp.add)
                nc.vector.tensor_reduce(sums_sb[rr, h], sar_sb[rr, h],
                                        axis=mybir.AxisListType.X, op=ALU.add)
                nc.vector.tensor_scalar_mul(chn_sb[rr, h, 0:2],
                                            sums_sb[rr, h, :, 0], 1.0 / NELEM)
                nc.vector.tensor_tensor(chn_sb[rr, h, 2:3], chn_sb[rr, h, 0:1],
                                        chn_sb[rr, h, 0:1], ALU.mult)
                nc.vector.tensor_tensor(chn_sb[rr, h, 3:4], chn_sb[rr, h, 1:2],
                                        chn_sb[rr, h, 2:3], ALU.subtract)
                nc.vector.tensor_scalar(rst_sb[rr, h], chn_sb[rr, h, 3:4],
                                        EPS, -0.5, ALU.add, ALU.pow)
                nc.vector.tensor_tensor(Gb_sb[rr, hp:hp + 1], rst_sb[rr, h],
                                        gamp_sb[rr, hp:hp + 1], ALU.mult)
                nc.vector.scalar_tensor_tensor(
                    nB_sb[rr, hp:hp + 1], chn_sb[rr, h, 0:1],
                    Gb_sb[rr, hp:hp + 1], betp_sb[rr, hp:hp + 1],
                    ALU.mult, ALU.subtract)

            # ---- interleaved emission: proj h+1 then attention h ----
            proj_head(0, *load_w(0))
            for sc in range(4):
                proj_v(sc)
            for h in range(HL):
                if h + 1 < HL:
                    proj_head(h + 1, *load_w(h + 1))
                if h == 0:
                    for sc in range(4, 8):
                        proj_v(sc)
                if PHASE >= 2:
                    attn_head(h)

            if PHASE <= 1:
                for h in range(HL):
                    nc.sync.dma_start(y_d[ts(h, 128), 0:512],
                                      qc_sb[:, h, :].bitcast(F32))

            psum_cm.__exit__(None, None, None)
            spool_cm.__exit__(None, None, None)
            epool_cm.__exit__(None, None, None)
            wpool_cm.__exit__(None, None, None)
            proj_pools.__exit__(None, None, None)

            if PHASE >= 6:
                # ---- transpose + GN apply + final linear ----
                fpool_cm = tc.tile_pool(name="fin", bufs=1)
                fpool = fpool_cm.__enter__()
                fps_cm = tc.tile_pool(name="fpsum", bufs=1, space="PSUM")
                fps = fps_cm.__enter__()

                for hp in range(4):
                    tr = fps.tile([128, 8, 128], mdt, tag="tr", bufs=2)
                    for tc_i in range(8):
                        nc.tensor.transpose(tr[0:64, tc_i],
                                            w_sb[:, 2 * hp, tc_i], idn_sb)
                        nc.tensor.transpose(tr[64:128, tc_i],
                                            w_sb[:, 2 * hp + 1, tc_i], idn_sb)
                    for tc_i in range(8):
                        nc.vector.tensor_scalar(
                            onorm[:, hp, ts(tc_i, 128)], tr[:, tc_i],
                            Gb_sb[:, hp:hp + 1], nB_sb[:, hp:hp + 1],
                            ALU.mult, ALU.subtract)

                for tt_ in range(8 if PHASE >= 7 else 0):
                    for eh in range(2):
                        py = fps.tile([128, 512], F32, tag="fp", bufs=3)
                        for o in range(4):
                            mm(py, onorm[:, o, ts(tt_, 128)],
                               ow_sb[:, o, ts(eh, 512)],
                               start=(o == 0), stop=(o == 3))
                        yt = fpool.tile([128, 512], F32, tag="yt", bufs=3)
                        nc.scalar.copy(yt, py)
                        nc.sync.dma_start(y_d[ts(tt_, 128), ts(eh, 512)], yt)
                if PHASE == 6:
                    for g in range(4):
                        nc.sync.dma_start(
                            y_d[ts(g, 256), :].rearrange("(a p) t -> p (a t)", p=128),
                            onorm[:, g].bitcast(F32))
                fps_cm.__exit__(None, None, None)
                fpool_cm.__exit__(None, None, None)

    nc.finalize()
    return nc


_NC = None


def _get_nc():
    global _NC
    if _NC is None:
        _NC = build_module()
    return _NC


def _prep_core(c, x, eo, Wq_cat, Wk_cat, qb_cat, kb_cat, Vw, Vb, lam, gamr, betr,
               out_w, np_mdt):
    b, hg = divmod(c, 2)
    hs = slice(hg * 8, (hg + 1) * 8)

    def dev(a):
        return np.ascontiguousarray(a.astype(np_mdt))

    xT = x[b].T.reshape(8, 128, T).transpose(1, 0, 2)
    eT = eo[b].T.reshape(8, 128, T).transpose(1, 0, 2)
    wq = Wq_cat[hs].transpose(2, 0, 1).reshape(8, 128, HL, 128).transpose(1, 2, 0, 3)
    wk = Wk_cat[hs].transpose(2, 0, 1).reshape(8, 128, HL, 128).transpose(1, 2, 0, 3)
    wv = Vw[hs].reshape(512, E).T.reshape(8, 128, 512).transpose(1, 0, 2)
    ow = out_w[:, hg * 512:(hg + 1) * 512].T.reshape(4, 128, T).transpose(1, 0, 2)
    # pair-packed gamma/beta: rows 0:64 = even head of pair, 64:128 = odd
    gamp = np.concatenate([gamr[hs][0::2].T, gamr[hs][1::2].T], axis=0)  # [128,4]
    betp = np.concatenate([betr[hs][0::2].T, betr[hs][1::2].T], axis=0)
    return {
        "xT": dev(xT),
        "eT": dev(eT),
        "wq": dev(wq),
        "wk": dev(wk),
        "wv": dev(wv),
        "qb": np.ascontiguousarray(qb_cat[hs].T, dtype=np.float32),
        "kb": np.ascontiguousarray(kb_cat[hs].T, dtype=np.float32),
        "vb": np.ascontiguousarray(np.tile(Vb[hs].reshape(1, 512), (128, 1)),
                                   dtype=np.float32),
        "lamb": np.ascontiguousarray(np.tile(lam[hs][None, :], (128, 1)),
                                     dtype=np.float32),
        "gamp": np.ascontiguousarray(gamp, dtype=np.float32),
        "betp": np.ascontiguousarray(betp, dtype=np.float32),
        "ow": dev(ow),
    }


def make_in_maps(inputs):
    x = np.asarray(inputs["x"], np.float32)
    eo = np.asarray(inputs["encoder_out"], np.float32)
    Wq_cat = np.concatenate([np.asarray(inputs["Q1w"], np.float32),
                             np.asarray(inputs["Q2w"], np.float32)], axis=1)
    Wk_cat = np.concatenate([np.asarray(inputs["K1w"], np.float32),
                             np.asarray(inputs["K2w"], np.float32)], axis=1)
    qb_cat = np.concatenate([np.asarray(inputs["Q1b"], np.float32),
                             np.asarray(inputs["Q2b"], np.float32)], axis=1)
    kb_cat = np.concatenate([np.asarray(inputs["K1b"], np.float32),
                             np.asarray(inputs["K2b"], np.float32)], axis=1)
    Vw = np.asarray(inputs["Vw"], np.float32)
    Vb = np.asarray(inputs["Vb"], np.float32)
    lam = np.asarray(inputs["lam"], np.float32)
    gamr = np.asarray(inputs["gn_gamma"], np.float32).reshape(H, DH)
    betr = np.asarray(inputs["gn_beta"], np.float32).reshape(H, DH)
    out_w = np.asarray(inputs["out_w"], np.float32)

    import ml_dtypes
    np_mdt = ml_dtypes.bfloat16

    maps = [
        _prep_core(c, x, eo, Wq_cat, Wk_cat, qb_cat, kb_cat, Vw, Vb, lam,
                   gamr, betr, out_w, np_mdt)
        for c in range(8)
    ]
    for m in maps:
        m["vone"] = np.ones((128, 8, HL, 1), np_mdt)
        m["idn"] = np.eye(128, dtype=np_mdt)
    return maps


def kernel(**inputs):
    global LAST_EXEC_NS, LAST_RESULTS
    nc = _get_nc()
    in_maps = make_in_maps(inputs)
    res = run_bass_kernel_spmd(nc, in_maps, core_ids=list(range(8)))
    LAST_EXEC_NS = res.exec_time_ns
    LAST_RESULTS = res
    out_b = np.asarray(inputs["out_b"], np.float32)
    parts = [res.results[c]["y"] for c in range(8)]
    y = np.stack([parts[2 * b] + parts[2 * b + 1] for b in range(B)])
    y = y + out_b[None, None, :]
    return y.astype(np.float32)


# revision 18
# speedup vs baseline: 1.0314x; 1.0147x over previous
"""Differential multi-head cross-attention Trainium2 kernel (v2).

Sharding: 8 cores = 4 batches x 2 head-groups (8 heads each). Each core
computes its (batch, head-group) shard fully on device; the host sums the
two head-group partials per batch and adds the output bias.

v2 dataflow (all matmul operands bf16):
  qcT/kcT [d_cat=128, T] per head (q1/k1 rows 0-63, q2/k2 rows 64-127)
  scores psum [128 s, 2 sign, 512 t]; ONE 1024-wide exp per (sc, th)
  po FLIPPED: poT[t, 65] = e[s,t].T @ v_aug[s, 65]  (col 64 = Z)
    -> per-partition Z: reciprocal + tensor_scalar ops, no broadcasts
  w^T[t,d] = po1T*r1 - po2T*(lam*r2)  via ts + stt (accum_out = row sums)
  sumsq via tensor_tensor_reduce; partition sums via gpsimd all-reduce
  rsqrt via DVE pow(-0.5); ScalarE runs ONLY Exp/Copy (one act table)
  PE transpose w^T -> [d, t]; GN scale/bias fused into psum->sbuf copy
  y_part[t, e] = onorm.T @ out_w_slice
"""
import os
import sys

if "/opt/trn_rl_repo" not in sys.path:
    sys.path.insert(0, "/opt/trn_rl_repo")

import numpy as np

import concourse.bass as bass
import concourse.mybir as mybir
import concourse.tile as tile
from concourse import bacc
from concourse.bass import ts
from concourse.bass_utils import run_bass_kernel_spmd

F32 = mybir.dt.float32
BF16 = mybir.dt.bfloat16
FP8 = mybir.dt.float8e4
AF = mybir.ActivationFunctionType
ALU = mybir.AluOpType
DR = mybir.MatmulPerfMode.DoubleRow

SCORES_FP8 = False  # fp8e4m3 q/k + DoubleRow score matmuls (fails 2e-2 gate)

B, T, E, H, DH = 4, 1024, 1024, 16, 64
HL = 8          # local heads per core
EPS = 1e-5
NELEM = float(T * DH)   # groupnorm element count per (b, h)

MM_MODE = "bf16"
# Debug: stop after phase N (1=proj, 3=+scores/exp/po, 4=+wchain, 7=full)
PHASE = int(os.environ.get("KPHASE", "70"))

LAST_EXEC_NS = None
LAST_RESULTS = None


def build_module():
    nc = bacc.Bacc("TRN2", target_bir_lowering=False, debug=False, num_devices=8)
    mdt = BF16

    xT_d = nc.declare_dram_parameter("xT", [128, 8, T], mdt, isOutput=False)
    eT_d = nc.declare_dram_parameter("eT", [128, 8, T], mdt, isOutput=False)
    wq_d = nc.declare_dram_parameter("wq", [128, HL, 8, 128], mdt, isOutput=False)
    wk_d = nc.declare_dram_parameter("wk", [128, HL, 8, 128], mdt, isOutput=False)
    wv_d = nc.declare_dram_parameter("wv", [128, 8, 512], mdt, isOutput=False)
    qb_d = nc.declare_dram_parameter("qb", [128, HL], F32, isOutput=False)
    kb_d = nc.declare_dram_parameter("kb", [128, HL], F32, isOutput=False)
    vb_d = nc.declare_dram_parameter("vb", [128, 512], F32, isOutput=False)
    lamb_d = nc.declare_dram_parameter("lamb", [128, HL], F32, isOutput=False)
    gamp_d = nc.declare_dram_parameter("gamp", [128, 4], F32, isOutput=False)
    betp_d = nc.declare_dram_parameter("betp", [128, 4], F32, isOutput=False)
    ow_d = nc.declare_dram_parameter("ow", [128, 4, T], mdt, isOutput=False)
    vone_d = nc.declare_dram_parameter("vone", [128, 8, HL, 1], mdt, isOutput=False)
    idn_d = nc.declare_dram_parameter("idn", [128, 128], mdt, isOutput=False)
    y_d = nc.declare_dram_parameter("y", [T, E], F32, isOutput=True)

    def mm(ps, lhsT, rhs, start, stop):
        nc.tensor.matmul(ps, lhsT, rhs, start=start, stop=stop)

    with tile.TileContext(nc) as tc:
        with (
            tc.tile_pool(name="const", bufs=1) as cpool,
        ):
            # ---- constants / small tensors ----
            qb_sb = cpool.tile([128, HL], F32)
            nc.sync.dma_start(qb_sb, qb_d[:])
            kb_sb = cpool.tile([128, HL], F32)
            nc.sync.dma_start(kb_sb, kb_d[:])
            vb_sb = cpool.tile([128, 512], F32)
            nc.sync.dma_start(vb_sb, vb_d[:])
            lamb_sb = cpool.tile([128, HL], F32)
            nc.sync.dma_start(lamb_sb, lamb_d[:])
            gamp_sb = cpool.tile([128, 4], F32)
            nc.sync.dma_start(gamp_sb, gamp_d[:])
            betp_sb = cpool.tile([128, 4], F32)
            nc.sync.dma_start(betp_sb, betp_d[:])
            idn_sb = cpool.tile([128, 128], mdt)
            nc.sync.dma_start(idn_sb, idn_d[:])

            # Prime engine vector-clocks on the const DMAs so later compute
            # instructions don't each accumulate DMA-queue waits (ISA caps
            # the sync-wait count per instruction).
            prime_a = cpool.tile([128, 1], F32)
            nc.scalar.copy(prime_a, qb_sb[:, 0:1])
            nc.scalar.copy(prime_a, kb_sb[:, 0:1])
            prime_d = cpool.tile([128, 1], F32)
            nc.vector.tensor_copy(prime_d, vb_sb[:, 0:1])
            nc.vector.tensor_copy(prime_d, lamb_sb[:, 0:1])
            nc.vector.tensor_copy(prime_d, gamp_sb[:, 0:1])
            nc.vector.tensor_copy(prime_d, betp_sb[:, 0:1])
            prime_g = cpool.tile([128, 1], F32)
            nc.gpsimd.tensor_copy(prime_g, betp_sb[:, 0:1])

            # ---- persistent big tensors ----
            v_sb = cpool.tile([128, 8, HL, 65], mdt)     # [s_part, s_chunk, h, d|1]
            nc.sync.dma_start(v_sb[:, :, :, 64:65], vone_d[:])
            qk_dt = FP8 if SCORES_FP8 else mdt
            qc_sb = cpool.tile([128, HL, T], qk_dt)
            kc_sb = cpool.tile([128, HL, T], qk_dt)
            onorm = cpool.tile([128, 4, T], mdt)
            ow_sb = cpool.tile([128, 4, T], mdt)
            w_sb = cpool.tile([128, HL, 8, 64], mdt)     # [t_part, h, tc, d]
            stat_sb = cpool.tile([128, HL, 2, 8], F32)   # [t_part, h, {sum,sq}, tc]
            sar_sb = cpool.tile([128, HL, 2, 8], F32)    # all-reduced stats
            sums_sb = cpool.tile([128, HL, 2, 1], F32)
            chn_sb = cpool.tile([128, HL, 4], F32)       # mean, m2, mean^2, var
            rst_sb = cpool.tile([128, HL, 1], F32)       # rstd
            Gb_sb = cpool.tile([128, 4], F32)            # per-pair scale
            nB_sb = cpool.tile([128, 4], F32)            # per-pair mean*G - beta

            proj_pools = tc.tile_pool(name="bigin", bufs=1)
            bpool = proj_pools.__enter__()
            wpool_cm = tc.tile_pool(name="wts", bufs=1)
            wpool = wpool_cm.__enter__()
            epool_cm = tc.tile_pool(name="eps", bufs=1)
            epool = epool_cm.__enter__()
            spool_cm = tc.tile_pool(name="small", bufs=1)
            spool = spool_cm.__enter__()
            psum_cm = tc.tile_pool(name="psum", bufs=1, space="PSUM")
            psum = psum_cm.__enter__()

            xT_sb = bpool.tile([128, 8, T], mdt, tag="xT")
            eT_sb = bpool.tile([128, 8, T], mdt, tag="eT")
            wv_sb = bpool.tile([128, 8, 512], mdt, tag="wv")
            wq0 = wpool.tile([128, 8, 128], mdt, tag="wq", bufs=2)
            nc.sync.dma_start(wq0, wq_d[:, 0])
            wk0 = wpool.tile([128, 8, 128], mdt, tag="wk", bufs=2)
            nc.sync.dma_start(wk0, wk_d[:, 0])
            for o in range(8):
                nc.sync.dma_start(xT_sb[:, o], xT_d[:, o])
            # eT and the bulk weights stream on the (idle) Pool DMA queue so
            # they don't serialize behind xT on the SP issue path.
            for o in range(8):
                nc.gpsimd.dma_start(eT_sb[:, o], eT_d[:, o])
            nc.gpsimd.dma_start(wv_sb, wv_d[:])
            nc.gpsimd.dma_start(ow_sb, ow_d[:])

            def proj_head(h, wq_t, wk_t):
                """q/k projections for head h (bf16 out with bias)."""
                for th in range(2):
                    pq = psum.tile([128, 512], F32, tag="pp", bufs=2)
                    for o in range(8):
                        mm(pq, wq_t[:, o], xT_sb[:, o, ts(th, 512)],
                           start=(o == 0), stop=(o == 7))
                    nc.vector.tensor_scalar_add(qc_sb[:, h, ts(th, 512)], pq,
                                                qb_sb[:, h:h + 1])
                    pk = psum.tile([128, 512], F32, tag="pp", bufs=2)
                    for o in range(8):
                        mm(pk, wk_t[:, o], eT_sb[:, o, ts(th, 512)],
                           start=(o == 0), stop=(o == 7))
                    nc.vector.tensor_scalar_add(kc_sb[:, h, ts(th, 512)], pk,
                                                kb_sb[:, h:h + 1])

            def proj_v(sc):
                """v projection chunk sc (natural [s, hd] layout)."""
                pv = psum.tile([128, 512], F32, tag="pp", bufs=2)
                for o in range(8):
                    mm(pv, eT_sb[:, o, ts(sc, 128)], wv_sb[:, o],
                       start=(o == 0), stop=(o == 7))
                nc.vector.tensor_tensor(
                    v_sb[:, sc, :, 0:64],
                    pv.rearrange("p (h d) -> p h d", d=64),
                    vb_sb.rearrange("p (h d) -> p h d", d=64),
                    ALU.add,
                )

            def load_w(h):
                if h == 0:
                    return wq0, wk0
                wq_t = wpool.tile([128, 8, 128], mdt, tag="wq", bufs=2)
                nc.gpsimd.dma_start(wq_t, wq_d[:, h])
                wk_t = wpool.tile([128, 8, 128], mdt, tag="wk", bufs=2)
                nc.gpsimd.dma_start(wk_t, wk_d[:, h])
                return wq_t, wk_t

            def attn_head(h):
                """scores + exp + flipped po + w-chain + stats for head h."""
                e_ts = []
                for th in range(2):
                    e_t = epool.tile([128, 8, 2, 512], mdt, tag="e", bufs=3)
                    e_ts.append(e_t)
                    for sc in range(8):
                        ps = psum.tile([128, 2, 512], F32, tag="sc", bufs=2)
                        if SCORES_FP8:
                            # DoubleRow with both k-tiles aliased (stride-0)
                            # computes 2*(k.T @ q); exp scale absorbs the 2x.
                            for sg in range(2):
                                pr = slice(64 * sg, 64 * sg + 64)
                                kb_ap = kc_sb[pr, h, ts(sc, 128)] \
                                    .unsqueeze(1).broadcast_to([64, 2, 128])
                                for tq in range(2):
                                    t0 = th * 512 + tq * 256
                                    qb_ap = qc_sb[pr, h, t0:t0 + 256] \
                                        .unsqueeze(1).broadcast_to([64, 2, 256])
                                    nc.tensor.matmul(
                                        ps[:, sg, ts(tq, 256)], kb_ap, qb_ap,
                                        start=True, stop=True, perf_mode=DR)
                        else:
                            mm(ps[:, 0], kc_sb[0:64, h, ts(sc, 128)],
                               qc_sb[0:64, h, ts(th, 512)], start=True, stop=True)
                            mm(ps[:, 1], kc_sb[64:128, h, ts(sc, 128)],
                               qc_sb[64:128, h, ts(th, 512)], start=True, stop=True)
                        nc.scalar.activation(e_t[:, sc], ps, AF.Exp,
                                             scale=0.0625 if SCORES_FP8 else 0.125)
                if PHASE <= 2:
                    if h == 0:
                        nc.sync.dma_start(y_d[0:128, 0:512],
                                          e_ts[0][:, 0:1, :, :].bitcast(F32))
                    return
                for tc_i in range(8):
                    th, tj = divmod(tc_i, 4)
                    e_t = e_ts[th]
                    po = psum.tile([128, 2, 65], F32, tag="po", bufs=2)
                    for sg in range(2):
                        for sc in range(8):
                            mm(po[:, sg], e_t[:, sc, sg, ts(tj, 128)],
                               v_sb[:, sc, h], start=(sc == 0), stop=(sc == 7))
                    if PHASE <= 3:
                        if h == 0:
                            pd = spool.tile([128, 130], F32, tag="pd", bufs=2)
                            nc.vector.tensor_copy(
                                pd, po.rearrange("p a b -> p (a b)"))
                            nc.sync.dma_start(
                                y_d[ts(tc_i, 128), 0:130], pd)
                        continue
                    # w^T chain: r = 1/Z; m2 = po2*r2*lam; w = po1*r1 - m2
                    r = spool.tile([128, 2], F32, tag="r", bufs=2)
                    nc.vector.reciprocal(r, po[:, :, 64])
                    m2 = spool.tile([128, 64], F32, tag="m2", bufs=2)
                    nc.vector.tensor_scalar(m2, po[:, 1, 0:64], r[:, 1:2],
                                            lamb_sb[:, h:h + 1],
                                            ALU.mult, ALU.mult)
                    nc.vector.scalar_tensor_tensor(
                        w_sb[:, h, tc_i], po[:, 0, 0:64], r[:, 0:1], m2,
                        ALU.mult, ALU.subtract,
                        accum_out=stat_sb[:, h, 0, tc_i:tc_i + 1])
                    sqs = spool.tile([128, 64], mdt, tag="sqs", bufs=2)
                    nc.vector.tensor_tensor_reduce(
                        sqs, w_sb[:, h, tc_i], w_sb[:, h, tc_i], 1.0, 0.0,
                        ALU.mult, ALU.add,
                        accum_out=stat_sb[:, h, 1, tc_i:tc_i + 1])

                if PHASE <= 4:
                    if PHASE == 4:
                        nc.sync.dma_start(
                            y_d[ts(h, 128), 0:256],
                            w_sb[:, h].bitcast(F32).rearrange("p a b -> p (a b)"))
                    return

                # ---- groupnorm stats (per head) ----
                hp, hj = divmod(h, 2)
                rr = slice(64 * hj, 64 * hj + 64)
                nc.gpsimd.partition_all_reduce(
                    sar_sb[:, h], stat_sb[:, h], channels=128,
                    reduce_op=bass.bass_isa.ReduceOp.add)
                nc.vector.tensor_reduce(sums_sb[rr, h], sar_sb[rr, h],
                                        axis=mybir.AxisListType.X, op=ALU.add)
                nc.vector.tensor_scalar_mul(chn_sb[rr, h, 0:2],
                                            sums_sb[rr, h, :, 0], 1.0 / NELEM)
                nc.vector.tensor_tensor(chn_sb[rr, h, 2:3], chn_sb[rr, h, 0:1],
                                        chn_sb[rr, h, 0:1], ALU.mult)
                nc.vector.tensor_tensor(chn_sb[rr, h, 3:4], chn_sb[rr, h, 1:2],
                                        chn_sb[rr, h, 2:3], ALU.subtract)
                nc.vector.tensor_scalar(rst_sb[rr, h], chn_sb[rr, h, 3:4],
                                        EPS, -0.5, ALU.add, ALU.pow)
                nc.vector.tensor_tensor(Gb_sb[rr, hp:hp + 1], rst_sb[rr, h],
                                        gamp_sb[rr, hp:hp + 1], ALU.mult)
                nc.vector.scalar_tensor_tensor(
                    nB_sb[rr, hp:hp + 1], chn_sb[rr, h, 0:1],
                    Gb_sb[rr, hp:hp + 1], betp_sb[rr, hp:hp + 1],
                    ALU.mult, ALU.subtract)

            # ---- interleaved emission: proj h+1 then attention h ----
            proj_head(0, *load_w(0))
            for sc in range(4):
                proj_v(sc)
            for h in range(HL):
                if h + 1 < HL:
                    proj_head(h + 1, *load_w(h + 1))
                if h == 0:
                    for sc in range(4, 8):
                        proj_v(sc)
                if PHASE >= 2:
                    attn_head(h)

            if PHASE <= 1:
                for h in range(HL):
                    nc.sync.dma_start(y_d[ts(h, 128), 0:512],
                                      qc_sb[:, h, :].bitcast(F32))

            psum_cm.__exit__(None, None, None)
            spool_cm.__exit__(None, None, None)
            epool_cm.__exit__(None, None, None)
            wpool_cm.__exit__(None, None, None)
            proj_pools.__exit__(None, None, None)

            if PHASE >= 6:
                # ---- transpose + GN apply + final linear ----
                fpool_cm = tc.tile_pool(name="fin", bufs=1)
                fpool = fpool_cm.__enter__()
                fps_cm = tc.tile_pool(name="fpsum", bufs=1, space="PSUM")
                fps = fps_cm.__enter__()

                for hp in range(4):
                    tr = fps.tile([128, 8, 128], mdt, tag="tr", bufs=2)
                    for tc_i in range(8):
                        nc.tensor.transpose(tr[0:64, tc_i],
                                            w_sb[:, 2 * hp, tc_i], idn_sb)
                        nc.tensor.transpose(tr[64:128, tc_i],
                                            w_sb[:, 2 * hp + 1, tc_i], idn_sb)
                    for tc_i in range(8):
                        nc.vector.tensor_scalar(
                            onorm[:, hp, ts(tc_i, 128)], tr[:, tc_i],
                            Gb_sb[:, hp:hp + 1], nB_sb[:, hp:hp + 1],
                            ALU.mult, ALU.subtract)

                for tt_ in range(8 if PHASE >= 7 else 0):
                    for eh in range(2):
                        py = fps.tile([128, 512], F32, tag="fp", bufs=3)
                        for o in range(4):
                            mm(py, onorm[:, o, ts(tt_, 128)],
                               ow_sb[:, o, ts(eh, 512)],
                               start=(o == 0), stop=(o == 3))
                        yt = fpool.tile([128, 512], F32, tag="yt", bufs=3)
                        nc.scalar.copy(yt, py)
                        nc.sync.dma_start(y_d[ts(tt_, 128), ts(eh, 512)], yt)
                if PHASE == 6:
                    for g in range(4):
                        nc.sync.dma_start(
                            y_d[ts(g, 256), :].rearrange("(a p) t -> p (a t)", p=128),
                            onorm[:, g].bitcast(F32))
                fps_cm.__exit__(None, None, None)
                fpool_cm.__exit__(None, None, None)

    nc.finalize()
    return nc


_NC = None


def _get_nc():
    global _NC
    if _NC is None:
        _NC = build_module()
    return _NC


def _prep_core(c, x, eo, Wq_cat, Wk_cat, qb_cat, kb_cat, Vw, Vb, lam, gamr, betr,
               out_w, np_mdt):
    b, hg = divmod(c, 2)
    hs = slice(hg * 8, (hg + 1) * 8)

    def dev(a):
        return np.ascontiguousarray(a.astype(np_mdt))

    xT = x[b].T.reshape(8, 128, T).transpose(1, 0, 2)
    eT = eo[b].T.reshape(8, 128, T).transpose(1, 0, 2)
    wq = Wq_cat[hs].transpose(2, 0, 1).reshape(8, 128, HL, 128).transpose(1, 2, 0, 3)
    wk = Wk_cat[hs].transpose(2, 0, 1).reshape(8, 128, HL, 128).transpose(1, 2, 0, 3)
    wv = Vw[hs].reshape(512, E).T.reshape(8, 128, 512).transpose(1, 0, 2)
    ow = out_w[:, hg * 512:(hg + 1) * 512].T.reshape(4, 128, T).transpose(1, 0, 2)
    # pair-packed gamma/beta: rows 0:64 = even head of pair, 64:128 = odd
    gamp = np.concatenate([gamr[hs][0::2].T, gamr[hs][1::2].T], axis=0)  # [128,4]
    betp = np.concatenate([betr[hs][0::2].T, betr[hs][1::2].T], axis=0)
    return {
        "xT": dev(xT),
        "eT": dev(eT),
        "wq": dev(wq),
        "wk": dev(wk),
        "wv": dev(wv),
        "qb": np.ascontiguousarray(qb_cat[hs].T, dtype=np.float32),
        "kb": np.ascontiguousarray(kb_cat[hs].T, dtype=np.float32),
        "vb": np.ascontiguousarray(np.tile(Vb[hs].reshape(1, 512), (128, 1)),
                                   dtype=np.float32),
        "lamb": np.ascontiguousarray(np.tile(lam[hs][None, :], (128, 1)),
                                     dtype=np.float32),
        "gamp": np.ascontiguousarray(gamp, dtype=np.float32),
        "betp": np.ascontiguousarray(betp, dtype=np.float32),
        "ow": dev(ow),
    }


def make_in_maps(inputs):
    x = np.asarray(inputs["x"], np.float32)
    eo = np.asarray(inputs["encoder_out"], np.float32)
    Wq_cat = np.concatenate([np.asarray(inputs["Q1w"], np.float32),
                             np.asarray(inputs["Q2w"], np.float32)], axis=1)
    Wk_cat = np.concatenate([np.asarray(inputs["K1w"], np.float32),
                             np.asarray(inputs["K2w"], np.float32)], axis=1)
    qb_cat = np.concatenate([np.asarray(inputs["Q1b"], np.float32),
                             np.asarray(inputs["Q2b"], np.float32)], axis=1)
    kb_cat = np.concatenate([np.asarray(inputs["K1b"], np.float32),
                             np.asarray(inputs["K2b"], np.float32)], axis=1)
    Vw = np.asarray(inputs["Vw"], np.float32)
    Vb = np.asarray(inputs["Vb"], np.float32)
    lam = np.asarray(inputs["lam"], np.float32)
    gamr = np.asarray(inputs["gn_gamma"], np.float32).reshape(H, DH)
    betr = np.asarray(inputs["gn_beta"], np.float32).reshape(H, DH)
    out_w = np.asarray(inputs["out_w"], np.float32)

    import ml_dtypes
    np_mdt = ml_dtypes.bfloat16

    maps = [
        _prep_core(c, x, eo, Wq_cat, Wk_cat, qb_cat, kb_cat, Vw, Vb, lam,
                   gamr, betr, out_w, np_mdt)
        for c in range(8)
    ]
    for m in maps:
        m["vone"] = np.ones((128, 8, HL, 1), np_mdt)
        m["idn"] = np.eye(128, dtype=np_mdt)
    return maps


def kernel(**inputs):
    global LAST_EXEC_NS, LAST_RESULTS
    nc = _get_nc()
    in_maps = make_in_maps(inputs)
    res = run_bass_kernel_spmd(nc, in_maps, core_ids=list(range(8)))
    LAST_EXEC_NS = res.exec_time_ns
    LAST_RESULTS = res
    out_b = np.asarray(inputs["out_b"], np.float32)
    parts = [res.results[c]["y"] for c in range(8)]
    y = np.stack([parts[2 * b] + parts[2 * b + 1] for b in range(B)])
    y = y + out_b[None, None, :]
    return y.astype(np.float32)
